# revision 1
# baseline (speedup 1.0000x reference)
"""Causal single-head attention on 8 trn2 cores, batch-data-parallel.

Computes, for each batch item b:
    Q = x[b] @ Wq + bq; K = x[b] @ Wk + bk; V = x[b] @ Wv + bv
    out[b] = softmax(causal_mask(Q K^T / sqrt(H))) @ V

Shapes: x [256, 256, 384], W* [384, 64], b* [64], out [256, 256, 64] fp32.
Sharding: batch axis split across 8 cores (32 items each), weights replicated.
The host feeds x transposed ([C, B*T] layout) so the kernel needs no on-device
transposition of x (contraction dim C must sit on SBUF partitions).

All matmul operands are float32r (TF32-like rounded fp32, ~1.6e-4 matmul rel
err, 4x PE throughput). Batch items are processed in pairs so the projection
matmuls stream N=512.

Per pair:
  qv psum [128,512] = rows 0:64 Q^T, rows 64:128 V^T (lhsT=[Wq|Wv], rhs=x^T)
  k  psum [64,512]  = K^T
  qv2/k2 sbuf = psum + per-partition bias ([bq;bv] and [bk]) -> one op each
Per batch item in the pair:
  V' = [V | 1 | 1] natural layout via PE transposes of V^T; the ones columns
       make the out matmul also produce the softmax denominator (wei @ 1)
  sT psum = scores^T: lhsT = K^T s-chunk, rhs = Q^T  (both h-major)
  W = exp(SCALE*sT + mask^T) -> written straight to SBUF, IS wei^T = out lhsT
  out' = wei^T.T @ V' -> [t, 64 | den | den];  out = out'[:,0:64] * (1/den)
"""

import numpy as np

import concourse.bacc as bacc
import concourse.mybir as mybir
import concourse.tile as tile
from concourse import bass_utils
from concourse.masks import make_identity

N_CORES = 8
B_FULL, T, C, H = 256, 256, 384, 64
B_SHARD = B_FULL // N_CORES  # 32
F32 = mybir.dt.float32
F32R = mybir.dt.float32r
SCALE = float(H) ** -0.5  # folded into exp: wei = exp(SCALE * scores + mask)
MASK_VAL = -1e30

ADD = mybir.AluOpType.add
MULT = mybir.AluOpType.mult
EXP = mybir.ActivationFunctionType.Exp


def _build():
    MMT = F32R
    nc = bacc.Bacc("TRN2", target_bir_lowering=False, debug=False, num_devices=N_CORES)

    # f32r-declared inputs: fp32 bit patterns fed directly; the PE's single-pass
    # fp32 mode consumes the high mantissa bits (same precision class as f32r
    # rounding, ~1e-4), and same-dtype DMA keeps loads on the fast HWDGE path.
    xT_d = nc.dram_tensor("xT", [C, B_SHARD * T], F32R, kind="ExternalInput").ap()
    wq_d = nc.dram_tensor("wq", [C, H], F32R, kind="ExternalInput").ap()
    wk_d = nc.dram_tensor("wk", [C, H], F32R, kind="ExternalInput").ap()
    wv_d = nc.dram_tensor("wv", [C, H], F32R, kind="ExternalInput").ap()
    bq_d = nc.dram_tensor("bq", [H, 1], F32, kind="ExternalInput").ap()
    bk_d = nc.dram_tensor("bk", [H, 1], F32, kind="ExternalInput").ap()
    bv_d = nc.dram_tensor("bv", [H, 1], F32, kind="ExternalInput").ap()
    out_d = nc.dram_tensor("out", [B_SHARD * T, H], F32, kind="ExternalOutput").ap()

    # x^T per batch pair: [p=c%128, k=c//128, t2=512]
    xT_r = xT_d.rearrange("(k p) (b t) -> b p k t", p=128, b=B_SHARD // 2)
    # out: t = n*128 + p per batch item
    out_r = out_d.rearrange("(b n p) h -> b p n h", p=128, n=2)

    with tile.TileContext(nc) as tc:
        with (
            tc.tile_pool(name="singles", bufs=1) as singles,
            tc.tile_pool(name="sb", bufs=3) as sb,
            tc.tile_pool(name="sbx", bufs=2) as sbx,
            tc.tile_pool(name="ps_qv", bufs=2, space="PSUM") as ps_qv,
            tc.tile_pool(name="ps_k", bufs=1, space="PSUM") as ps_k,
            tc.tile_pool(name="ps_s", bufs=2, space="PSUM") as ps_s,
            tc.tile_pool(name="ps_v", bufs=1, space="PSUM") as ps_v,
            tc.tile_pool(name="ps_o", bufs=2, space="PSUM") as ps_o,
        ):
            # ---- one-time setup ----
            identf = singles.tile([128, 128], F32)
            make_identity(nc, identf[:])
            ident = singles.tile([128, 128], MMT)
            nc.vector.tensor_copy(ident[:], identf[:])

            # mask for scores^T [s, t]: keep where t >= s, two diag blocks
            maskT2 = singles.tile([128, 2, 128], F32)
            nc.gpsimd.memset(maskT2[:], 0.0)
            for j in range(2):
                nc.gpsimd.affine_select(
                    out=maskT2[:, j, :],
                    in_=maskT2[:, j, :],
                    compare_op=mybir.AluOpType.is_ge,
                    fill=MASK_VAL,
                    base=0,
                    pattern=[[1, 128]],  # keep where (-s + t) >= 0
                    channel_multiplier=-1,
                )

            # [Wq | Wv] stacked along M; Wk zero-padded to M=128 (f32r matmuls
            # with partial column groups run in a slower mode).
            wqv = singles.tile([128, 3, 128], MMT)
            wkk = singles.tile([128, 3, 128], MMT)
            nc.vector.memset(wkk[:].bitcast(F32), 0.0)
            for c in range(3):
                nc.sync.dma_start(wqv[:, c, 0:64], wq_d[c * 128 : (c + 1) * 128, :])
                nc.sync.dma_start(wqv[:, c, 64:128], wv_d[c * 128 : (c + 1) * 128, :])
                nc.sync.dma_start(wkk[:, c, 0:64], wk_d[c * 128 : (c + 1) * 128, :])
            # per-partition bias vectors: [bq ; bv] and [bk]
            bqv_t = singles.tile([128, 1], F32)
            bk_t = singles.tile([64, 1], F32)
            nc.sync.dma_start(bqv_t[0:64, :], bq_d[:])
            nc.sync.dma_start(bqv_t[64:128, :], bv_d[:])
            nc.sync.dma_start(bk_t[:], bk_d[:])

            # HAM warmup: the PE clock-gate only opens (1.2 -> 2.4 GHz) after a
            # ~3.4us window of sustained matmul activity. Burn dummy matmuls
            # during the initial DMA wait so the real stream runs warm.
            wu = singles.tile([128, 256], MMT)
            nc.vector.memset(wu[:].bitcast(F32), 0.0)
            wu_ps = ps_s.tile([128, 256], F32, tag="s_ps")
            for _ in range(64):
                nc.tensor.matmul(wu_ps[:], wu[:, 0:128], wu[:], start=True, stop=True)

            for bp in range(B_SHARD // 2):
                # x^T for the pair (f32r bits straight off HWDGE)
                xt = sbx.tile([128, 3, 512], MMT, tag="xt")
                nc.sync.dma_start(xt[:], xT_r[bp])

                # pair projections
                qv_ps = ps_qv.tile([128, 512], F32, tag="qv_ps")
                k_ps = ps_k.tile([128, 512], F32, tag="k_ps")
                for c in range(3):
                    nc.tensor.matmul(
                        qv_ps[:], wqv[:, c, :], xt[:, c, :], start=(c == 0), stop=(c == 2)
                    )
                for c in range(3):
                    nc.tensor.matmul(
                        k_ps[:], wkk[:, c, :], xt[:, c, :], start=(c == 0), stop=(c == 2)
                    )
                qv2 = sb.tile([128, 512], MMT, tag="qv2")
                k2 = sb.tile([64, 512], MMT, tag="k2")
                nc.vector.tensor_scalar_add(qv2[:], qv_ps[:], bqv_t[:])
                nc.vector.tensor_scalar_add(k2[:], k_ps[0:64, :], bk_t[:])

                for bi in range(2):
                    toff = bi * 256
                    qT = qv2[0:64, toff : toff + 256]
                    kT = k2[0:64, toff : toff + 256]

                    # scores^T [s, t]: blocks [s0, t0:256] and [s1, t0:256]
                    s_ps = ps_s.tile([128, 4, 128], F32, tag="s_ps")
                    nc.tensor.matmul(s_ps[:, 0:2, :], kT[:, 0:128], qT, start=True, stop=True)
                    nc.tensor.matmul(s_ps[:, 2:4, :], kT[:, 128:256], qT, start=True, stop=True)

                    # wei^T = exp(SCALE*scores^T + mask), straight to SBUF
                    E = sb.tile([128, 2, 128], F32, tag="E")
                    W = sb.tile([128, 3, 128], MMT, tag="W")
                    nc.vector.tensor_add(E[:], s_ps[:, 0:4:3, :], maskT2[:])
                    nc.scalar.activation(W[:, 0:3:2, :], E[:], EXP, scale=SCALE)
                    nc.scalar.activation(W[:, 1, :], s_ps[:, 1, :], EXP, scale=SCALE)

                    # V natural [s, h] + ones cols via PE transpose of V^T
                    v_ps = ps_v.tile([128, 2, 64], MMT, tag="v_ps")
                    for sh in range(2):
                        nc.tensor.transpose(
                            v_ps[:, sh, :],
                            qv2[64:128, toff + sh * 128 : toff + (sh + 1) * 128],
                            ident[64:128, 64:128],
                        )
                    v_sb = sb.tile([128, 2, 66], MMT, tag="v_sb")
                    nc.scalar.copy(v_sb[:, :, 0:64], v_ps[:])
                    nc.vector.memset(v_sb[:, :, 64:66].bitcast(F32), 1.0)

                    # out' = wei^T.T @ [V|1|1] -> [t, 64 | den | den]
                    o_ps = ps_o.tile([128, 2, 66], F32, tag="o_ps")
                    nc.tensor.matmul(o_ps[:, 0, :], W[:, 0, :], v_sb[:, 0, :], start=True, stop=True)
                    nc.tensor.matmul(o_ps[:, 1, :], W[:, 1, :], v_sb[:, 0, :], start=True, stop=False)
                    nc.tensor.matmul(o_ps[:, 1, :], W[:, 2, :], v_sb[:, 1, :], start=False, stop=True)

                    rden = sb.tile([128, 2], F32, tag="rden")
                    nc.vector.reciprocal(rden[:], o_ps[:, :, 64])
                    o_sb = sb.tile([128, 2, 64], F32, tag="o_sb")
                    nc.vector.tensor_scalar_mul(o_sb[:, 0, :], o_ps[:, 0, 0:64], rden[:, 0:1])
                    nc.vector.tensor_scalar_mul(o_sb[:, 1, :], o_ps[:, 1, 0:64], rden[:, 1:2])
                    nc.sync.dma_start(out_r[bp * 2 + bi], o_sb[:])

    nc.compile()
    return nc


_CACHE = {}


def get_nc():
    if "nc" not in _CACHE:
        _CACHE["nc"] = _build()
    return _CACHE["nc"]


def make_in_maps(x, Wq, bq, Wk, bk, Wv, bv):
    x = np.asarray(x, dtype=np.float32)
    Wq = np.ascontiguousarray(np.asarray(Wq, dtype=np.float32))
    Wk = np.ascontiguousarray(np.asarray(Wk, dtype=np.float32))
    Wv = np.ascontiguousarray(np.asarray(Wv, dtype=np.float32))
    bq = np.ascontiguousarray(np.asarray(bq, dtype=np.float32)).reshape(H, 1)
    bk = np.ascontiguousarray(np.asarray(bk, dtype=np.float32)).reshape(H, 1)
    bv = np.ascontiguousarray(np.asarray(bv, dtype=np.float32)).reshape(H, 1)
    in_maps = []
    for i in range(N_CORES):
        shard = x[i * B_SHARD : (i + 1) * B_SHARD].reshape(B_SHARD * T, C)
        xT = np.ascontiguousarray(shard.T)  # [C, B_SHARD*T]
        in_maps.append(
            {"xT": xT, "wq": Wq, "wk": Wk, "wv": Wv, "bq": bq, "bk": bk, "bv": bv}
        )
    return in_maps


def kernel(x, Wq, bq, Wk, bk, Wv, bv):
    nc = get_nc()
    in_maps = make_in_maps(x, Wq, bq, Wk, bk, Wv, bv)
    res = bass_utils.run_bass_kernel_spmd(nc, in_maps, core_ids=list(range(N_CORES)))
    out = np.concatenate(
        [res.results[i]["out"].reshape(B_SHARD, T, H) for i in range(N_CORES)], axis=0
    )
    return out



# revision 3
# speedup vs baseline: 1.3489x; 1.3489x over previous
"""Causal single-head attention on 8 trn2 cores, batch-data-parallel.

Computes, for each batch item b:
    Q = x[b] @ Wq + bq; K = x[b] @ Wk + bk; V = x[b] @ Wv + bv
    out[b] = softmax(causal_mask(Q K^T / sqrt(H))) @ V

Shapes: x [256, 256, 384], W* [384, 64], b* [64], out [256, 256, 64] fp32.
Sharding: batch axis split across 8 cores (32 items each), weights replicated.
The host feeds x transposed ([C, B*T] layout) so the kernel needs no on-device
transposition of x (contraction dim C must sit on SBUF partitions).

All matmul operands are bfloat16 (PSUM accumulation stays fp32): bf16 runs
1 cycle/row on the PE at ANY moving-dim size (f32r pays 4x below 256), halves
DMA/SBUF traffic, and avoids the HAM half-clock throttle observed on dense
f32r streams. Measured rel err ~2e-3, well inside the 2e-2 gate.

Batch items are processed in pairs so the projection matmuls stream N=512.
Per pair:
  qv psum [128,512] = rows 0:64 Q^T, rows 64:128 V^T (lhsT=[Wq|Wv], rhs=x^T)
  k  psum [64,512]  = K^T
  qv2/k2 sbuf (bf16) = psum + per-partition bias -> one op each
Per batch item in the pair:
  V' = [V | 1 | 1] natural layout via PE transposes of V^T; the ones columns
       make the out matmul also produce the softmax denominator (wei @ 1)
  sT psum = scores^T: lhsT = K^T s-chunk, rhs = Q^T  (both h-major).
       Block (s1,t0) is causally dead and never computed (mm2 streams t1 only).
  W = exp(SCALE*sT + mask^T) -> bf16 straight to SBUF, IS wei^T = out lhsT
  out' = wei^T.T @ V' -> [t, 64 | den | den];  out = out'[:,0:64] * (1/den)
"""

import ml_dtypes
import numpy as np

import concourse.bacc as bacc
import concourse.mybir as mybir
import concourse.tile as tile
from concourse import bass_utils
from concourse.masks import make_identity

N_CORES = 8
B_FULL, T, C, H = 256, 256, 384, 64
B_SHARD = B_FULL // N_CORES  # 32
F32 = mybir.dt.float32
BF16 = mybir.dt.bfloat16
SCALE = float(H) ** -0.5  # folded into exp: wei = exp(SCALE * scores + mask)
MASK_VAL = -1e30

EXP = mybir.ActivationFunctionType.Exp


def _build():
    MMT = BF16
    nc = bacc.Bacc("TRN2", target_bir_lowering=False, debug=False, num_devices=N_CORES)

    xT_d = nc.dram_tensor("xT", [C, B_SHARD * T], MMT, kind="ExternalInput").ap()
    wq_d = nc.dram_tensor("wq", [C, H], MMT, kind="ExternalInput").ap()
    wk_d = nc.dram_tensor("wk", [C, H], MMT, kind="ExternalInput").ap()
    wv_d = nc.dram_tensor("wv", [C, H], MMT, kind="ExternalInput").ap()
    bq_d = nc.dram_tensor("bq", [H, 1], F32, kind="ExternalInput").ap()
    bk_d = nc.dram_tensor("bk", [H, 1], F32, kind="ExternalInput").ap()
    bv_d = nc.dram_tensor("bv", [H, 1], F32, kind="ExternalInput").ap()
    out_d = nc.dram_tensor("out", [B_SHARD * T, H], F32, kind="ExternalOutput").ap()

    # x^T per batch pair: [p=c%128, k=c//128, t2=512]
    xT_r = xT_d.rearrange("(k p) (b t) -> b p k t", p=128, b=B_SHARD // 2)
    # out: t = n*128 + p per batch item
    out_r = out_d.rearrange("(b n p) h -> b p n h", p=128, n=2)

    with tile.TileContext(nc) as tc:
        with (
            tc.tile_pool(name="singles", bufs=1) as singles,
            tc.tile_pool(name="sb", bufs=3) as sb,
            tc.tile_pool(name="sbx", bufs=2) as sbx,
            tc.tile_pool(name="ps_qv", bufs=2, space="PSUM") as ps_qv,
            tc.tile_pool(name="ps_k", bufs=1, space="PSUM") as ps_k,
            tc.tile_pool(name="ps_s", bufs=2, space="PSUM") as ps_s,
            tc.tile_pool(name="ps_v", bufs=1, space="PSUM") as ps_v,
            tc.tile_pool(name="ps_o", bufs=2, space="PSUM") as ps_o,
        ):
            # ---- one-time setup ----
            identf = singles.tile([128, 128], F32)
            make_identity(nc, identf[:])
            ident = singles.tile([128, 128], MMT)
            nc.vector.tensor_copy(ident[:], identf[:])

            # mask for scores^T [s, t]: keep where t >= s, two diag blocks
            maskT2 = singles.tile([128, 2, 128], F32)
            nc.gpsimd.memset(maskT2[:], 0.0)
            for j in range(2):
                nc.gpsimd.affine_select(
                    out=maskT2[:, j, :],
                    in_=maskT2[:, j, :],
                    compare_op=mybir.AluOpType.is_ge,
                    fill=MASK_VAL,
                    base=0,
                    pattern=[[1, 128]],  # keep where (-s + t) >= 0
                    channel_multiplier=-1,
                )

            # [Wq | Wv] stacked along M; Wk on its own pass (M=64 used).
            wqv = singles.tile([128, 3, 128], MMT)
            wkk = singles.tile([128, 3, 64], MMT)
            for c in range(3):
                nc.sync.dma_start(wqv[:, c, 0:64], wq_d[c * 128 : (c + 1) * 128, :])
                nc.sync.dma_start(wqv[:, c, 64:128], wv_d[c * 128 : (c + 1) * 128, :])
                nc.sync.dma_start(wkk[:, c, :], wk_d[c * 128 : (c + 1) * 128, :])
            # per-partition bias vectors: [bq ; bv] and [bk]
            bqv_t = singles.tile([128, 1], F32)
            bk_t = singles.tile([64, 1], F32)
            nc.sync.dma_start(bqv_t[0:64, :], bq_d[:])
            nc.sync.dma_start(bqv_t[64:128, :], bv_d[:])
            nc.sync.dma_start(bk_t[:], bk_d[:])

            for bp in range(B_SHARD // 2):
                # x^T for the pair
                xt = sbx.tile([128, 3, 512], MMT, tag="xt")
                nc.sync.dma_start(xt[:], xT_r[bp])

                # pair projections
                qv_ps = ps_qv.tile([128, 512], F32, tag="qv_ps")
                k_ps = ps_k.tile([64, 512], F32, tag="k_ps")
                for c in range(3):
                    nc.tensor.matmul(
                        qv_ps[:], wqv[:, c, :], xt[:, c, :], start=(c == 0), stop=(c == 2)
                    )
                for c in range(3):
                    nc.tensor.matmul(
                        k_ps[:], wkk[:, c, :], xt[:, c, :], start=(c == 0), stop=(c == 2)
                    )
                qv2 = sb.tile([128, 512], MMT, tag="qv2")
                k2 = sb.tile([64, 512], MMT, tag="k2")
                nc.vector.tensor_scalar_add(qv2[:], qv_ps[:], bqv_t[:])
                nc.vector.tensor_scalar_add(k2[:], k_ps[:], bk_t[:])

                for bi in range(2):
                    toff = bi * 256
                    qT = qv2[0:64, toff : toff + 256]
                    kT = k2[0:64, toff : toff + 256]

                    # scores^T [s, t] blocks: 0=(s0,t0) 1=(s0,t1) 2=(s1,t1);
                    # (s1,t0) is causally dead and skipped.
                    s_ps = ps_s.tile([128, 3, 128], F32, tag="s_ps")
                    nc.tensor.matmul(s_ps[:, 0:2, :], kT[:, 0:128], qT, start=True, stop=True)
                    nc.tensor.matmul(
                        s_ps[:, 2, :], kT[:, 128:256], qT[:, 128:256], start=True, stop=True
                    )

                    # wei^T = exp(SCALE*scores^T + mask), bf16 straight to SBUF
                    E = sb.tile([128, 2, 128], F32, tag="E")
                    W = sb.tile([128, 3, 128], MMT, tag="W")
                    nc.vector.tensor_add(E[:], s_ps[:, 0:3:2, :], maskT2[:])
                    nc.scalar.activation(W[:, 0:3:2, :], E[:], EXP, scale=SCALE)
                    nc.scalar.activation(W[:, 1, :], s_ps[:, 1, :], EXP, scale=SCALE)

                    # V natural [s, h] + ones cols via PE transpose of V^T
                    v_ps = ps_v.tile([128, 2, 64], MMT, tag="v_ps")
                    for sh in range(2):
                        nc.tensor.transpose(
                            v_ps[:, sh, :],
                            qv2[64:128, toff + sh * 128 : toff + (sh + 1) * 128],
                            ident[64:128, 64:128],
                        )
                    v_sb = sb.tile([128, 2, 66], MMT, tag="v_sb")
                    nc.scalar.copy(v_sb[:, :, 0:64], v_ps[:])
                    nc.vector.memset(v_sb[:, :, 64:66], 1.0)

                    # out' = wei^T.T @ [V|1|1] -> [t, 64 | den | den]
                    o_ps = ps_o.tile([128, 2, 66], F32, tag="o_ps")
                    nc.tensor.matmul(o_ps[:, 0, :], W[:, 0, :], v_sb[:, 0, :], start=True, stop=True)
                    nc.tensor.matmul(o_ps[:, 1, :], W[:, 1, :], v_sb[:, 0, :], start=True, stop=False)
                    nc.tensor.matmul(o_ps[:, 1, :], W[:, 2, :], v_sb[:, 1, :], start=False, stop=True)

                    rden = sb.tile([128, 2], F32, tag="rden")
                    nc.vector.reciprocal(rden[:], o_ps[:, :, 64])
                    o_sb = sb.tile([128, 2, 64], F32, tag="o_sb")
                    nc.vector.tensor_scalar_mul(o_sb[:, 0, :], o_ps[:, 0, 0:64], rden[:, 0:1])
                    nc.vector.tensor_scalar_mul(o_sb[:, 1, :], o_ps[:, 1, 0:64], rden[:, 1:2])
                    nc.sync.dma_start(out_r[bp * 2 + bi], o_sb[:])

    nc.compile()
    return nc


_CACHE = {}


def get_nc():
    if "nc" not in _CACHE:
        _CACHE["nc"] = _build()
    return _CACHE["nc"]


def make_in_maps(x, Wq, bq, Wk, bk, Wv, bv):
    bf16 = ml_dtypes.bfloat16
    x = np.asarray(x, dtype=np.float32)
    Wq = np.ascontiguousarray(np.asarray(Wq, dtype=bf16))
    Wk = np.ascontiguousarray(np.asarray(Wk, dtype=bf16))
    Wv = np.ascontiguousarray(np.asarray(Wv, dtype=bf16))
    bq = np.ascontiguousarray(np.asarray(bq, dtype=np.float32)).reshape(H, 1)
    bk = np.ascontiguousarray(np.asarray(bk, dtype=np.float32)).reshape(H, 1)
    bv = np.ascontiguousarray(np.asarray(bv, dtype=np.float32)).reshape(H, 1)
    in_maps = []
    for i in range(N_CORES):
        shard = x[i * B_SHARD : (i + 1) * B_SHARD].reshape(B_SHARD * T, C)
        xT = np.ascontiguousarray(shard.T.astype(bf16))  # [C, B_SHARD*T]
        in_maps.append(
            {"xT": xT, "wq": Wq, "wk": Wk, "wv": Wv, "bq": bq, "bk": bk, "bv": bv}
        )
    return in_maps


def kernel(x, Wq, bq, Wk, bk, Wv, bv):
    nc = get_nc()
    in_maps = make_in_maps(x, Wq, bq, Wk, bk, Wv, bv)
    res = bass_utils.run_bass_kernel_spmd(nc, in_maps, core_ids=list(range(N_CORES)))
    out = np.concatenate(
        [res.results[i]["out"].reshape(B_SHARD, T, H) for i in range(N_CORES)], axis=0
    )
    return out


# revision 6
# speedup vs baseline: 1.6288x; 1.2075x over previous
"""Causal single-head attention on 8 trn2 cores, batch-data-parallel.

Computes, for each batch item b:
    Q = x[b] @ Wq + bq; K = x[b] @ Wk + bk; V = x[b] @ Wv + bv
    out[b] = softmax(causal_mask(Q K^T / sqrt(H))) @ V

Shapes: x [256, 256, 384], W* [384, 64], b* [64], out [256, 256, 64] fp32.
Sharding: batch axis split across 8 cores (32 items each), weights replicated.

All matmul operands are bfloat16 (PSUM accumulation stays fp32); rel err
~4e-3 against the fp32 reference (gate 2e-2).

Layout choices driven by trace analysis:
- Host feeds x^T per batch pair as [pair, partition, 3KB contiguous] so each
  pair is ONE dma with 128 descriptors (descriptor generation on the Sync
  engine at ~5ns/descriptor was a startup+steady bottleneck).
- All matmul weights ship in ONE packed dram tensor (12 dma_starts at ~0.7us
  of descriptor-gen each delayed the first x tile by ~8us in v2).
- Output is written pair-at-a-time in a [pair, partition, item, half, h]
  layout (1KB/partition contiguous, one dma per pair); the host undoes it.

Per pair (batch items 2bp, 2bp+1):
  qv psum [128,512] = rows 0:64 Q^T, rows 64:128 V^T (lhsT=[Wq|Wv], rhs=x^T)
  k  psum [64,512]  = K^T
  qv2/k2 sbuf (bf16) = psum + per-partition bias
  scores^T psum [128, item, 3, 128]: per item blocks (s0,t0) (s0,t1) (s1,t1);
      the causally-dead (s1,t0) block is never computed.
  W = exp(SCALE*scores^T) in ONE activation op (psum -> bf16 sbuf);
      the causal mask is applied AFTER exp by zeroing the upper triangle of
      the two diagonal blocks per item with gpsimd affine_select (GpSimd has
      no PSUM port, but W is in SBUF; this keeps mask work off the busy DVE).
  V' = [V | 1 | 1] natural layout via PE transposes of V^T; the ones columns
       make the out matmul also produce the softmax denominator (wei @ 1)
  out' = wei^T.T @ V' -> [t, 64 | den | den];  out = out'[:,0:64] * (1/den)
"""

import ml_dtypes
import numpy as np

import concourse.bacc as bacc
import concourse.mybir as mybir
import concourse.tile as tile
from concourse import bass_utils
from concourse.masks import make_identity

N_CORES = 8
B_FULL, T, C, H = 256, 256, 384, 64
B_SHARD = B_FULL // N_CORES  # 32
NPAIR = B_SHARD // 2  # 16
F32 = mybir.dt.float32
BF16 = mybir.dt.bfloat16
SCALE = float(H) ** -0.5

EXP = mybir.ActivationFunctionType.Exp


def _build():
    MMT = BF16
    nc = bacc.Bacc("TRN2", target_bir_lowering=False, debug=False, num_devices=N_CORES)

    # x^T pair-major: row (bp*128+p), cols (k*512+t) -> x[pair bp][t][k*128+p]
    xT_d = nc.dram_tensor("xT", [NPAIR * 128, 3 * 512], MMT, kind="ExternalInput").ap()
    # all matmul weights in one blob: cols [c*128 .. c*128+64) = Wq chunk c,
    # [c*128+64 ..) = Wv chunk c, cols [384+c*64) = Wk chunk c
    wp_d = nc.dram_tensor("wpack", [128, 576], MMT, kind="ExternalInput").ap()
    # biases: col 0 = [bq;bv], col 1 = [bk;0]
    bias_d = nc.dram_tensor("bias", [128, 2], F32, kind="ExternalInput").ap()
    # out pair-major: row (bp*128+p), cols ((bi*2+n)*64+h) -> item 2bp+bi,
    # t = n*128+p
    out_d = nc.dram_tensor("out", [NPAIR * 128, 256], F32, kind="ExternalOutput").ap()

    xT_r = xT_d.rearrange("(b p) (k t) -> b p k t", p=128, k=3)
    out_r = out_d.rearrange("(b p) (i n h) -> b p i n h", p=128, i=2, n=2)

    with tile.TileContext(nc) as tc:
        with (
            tc.tile_pool(name="singles", bufs=1) as singles,
            tc.tile_pool(name="sb", bufs=3) as sb,
            tc.tile_pool(name="sbx", bufs=3) as sbx,
            tc.tile_pool(name="ps_qv", bufs=2, space="PSUM") as ps_qv,
            tc.tile_pool(name="ps_k", bufs=1, space="PSUM") as ps_k,
            tc.tile_pool(name="ps_s", bufs=2, space="PSUM") as ps_s,
            tc.tile_pool(name="ps_v", bufs=1, space="PSUM") as ps_v,
            tc.tile_pool(name="ps_o", bufs=2, space="PSUM") as ps_o,
        ):
            # ---- one-time setup ----
            wt = singles.tile([128, 576], MMT)
            nc.sync.dma_start(wt[:], wp_d[:])
            bias_t = singles.tile([128, 2], F32)
            nc.sync.dma_start(bias_t[:], bias_d[:])
            wqv = wt[:, 0:384].rearrange("p (c m) -> p c m", c=3)  # [128, 3, 128]
            wkk = wt[:, 384:576].rearrange("p (c m) -> p c m", c=3)  # [128, 3, 64]
            bqv_t = bias_t[:, 0:1]
            bk_t = bias_t[0:64, 1:2]

            identf = singles.tile([128, 128], F32)
            make_identity(nc, identf[:])
            ident = singles.tile([128, 128], MMT)
            nc.vector.tensor_copy(ident[:], identf[:])

            for bp in range(NPAIR):
                # x^T for the pair: [p, k, t] (3KB/partition, one dma)
                xt = sbx.tile([128, 3, 512], MMT, tag="xt")
                nc.sync.dma_start(xt[:], xT_r[bp])

                # pair projections
                qv_ps = ps_qv.tile([128, 512], F32, tag="qv_ps")
                k_ps = ps_k.tile([64, 512], F32, tag="k_ps")
                for c in range(3):
                    nc.tensor.matmul(
                        qv_ps[:], wqv[:, c, :], xt[:, c, :], start=(c == 0), stop=(c == 2)
                    )
                for c in range(3):
                    nc.tensor.matmul(
                        k_ps[:], wkk[:, c, :], xt[:, c, :], start=(c == 0), stop=(c == 2)
                    )
                qv2 = sb.tile([128, 512], MMT, tag="qv2")
                k2 = sb.tile([64, 512], MMT, tag="k2")
                nc.vector.tensor_scalar_add(qv2[:], qv_ps[:], bqv_t)
                nc.vector.tensor_scalar_add(k2[:], k_ps[:], bk_t)

                # scores^T per item: blocks 0=(s0,t0) 1=(s0,t1) 2=(s1,t1)
                W = sb.tile([128, 2, 3, 128], MMT, tag="W")
                for bi in range(2):
                    toff = bi * 256
                    qT = qv2[0:64, toff : toff + 256]
                    kT = k2[0:64, toff : toff + 256]
                    s_ps = ps_s.tile([128, 3, 128], F32, tag="s_ps")
                    nc.tensor.matmul(
                        s_ps[:, 0:2, :], kT[:, 0:128], qT, start=True, stop=True
                    )
                    nc.tensor.matmul(
                        s_ps[:, 2, :], kT[:, 128:256], qT[:, 128:256], start=True, stop=True
                    )
                    # wei^T = exp(SCALE*scores^T), bf16 straight to SBUF
                    nc.scalar.activation(W[:, bi, :, :], s_ps[:], EXP, scale=SCALE)
                    # causal mask: zero upper triangle of the diagonal blocks
                    for j in (0, 2):
                        nc.gpsimd.affine_select(
                            out=W[:, bi, j, :],
                            in_=W[:, bi, j, :],
                            compare_op=mybir.AluOpType.is_ge,
                            fill=0.0,
                            base=0,
                            pattern=[[1, 128]],  # keep where (-s + t) >= 0
                            channel_multiplier=-1,
                        )

                # V natural [s, h] for both items via PE transposes of V^T
                v_ps = ps_v.tile([128, 4, 64], MMT, tag="v_ps")
                for q in range(4):
                    nc.tensor.transpose(
                        v_ps[:, q, :],
                        qv2[64:128, q * 128 : (q + 1) * 128],
                        ident[64:128, 64:128],
                    )
                v_sb = sb.tile([128, 2, 2, 66], MMT, tag="v_sb")
                nc.scalar.copy(
                    v_sb[:, :, :, 0:64].rearrange("p i n h -> p (i n) h"), v_ps[:]
                )
                nc.vector.memset(v_sb[:, :, :, 64:66], 1.0)

                # out' = wei^T.T @ [V|1|1] -> [t, 64 | den | den]
                o_ps = ps_o.tile([128, 2, 2, 66], F32, tag="o_ps")
                for bi in range(2):
                    nc.tensor.matmul(
                        o_ps[:, bi, 0, :], W[:, bi, 0, :], v_sb[:, bi, 0, :],
                        start=True, stop=True,
                    )
                    nc.tensor.matmul(
                        o_ps[:, bi, 1, :], W[:, bi, 1, :], v_sb[:, bi, 0, :],
                        start=True, stop=False,
                    )
                    nc.tensor.matmul(
                        o_ps[:, bi, 1, :], W[:, bi, 2, :], v_sb[:, bi, 1, :],
                        start=False, stop=True,
                    )

                rden = sb.tile([128, 2, 2], F32, tag="rden")
                nc.vector.reciprocal(rden[:], o_ps[:, :, :, 64])
                o_sb = sb.tile([128, 2, 2, 64], F32, tag="o_sb")
                for bi in range(2):
                    for n in range(2):
                        nc.vector.tensor_scalar_mul(
                            o_sb[:, bi, n, :], o_ps[:, bi, n, 0:64], rden[:, bi, n : n + 1]
                        )
                nc.sync.dma_start(out_r[bp], o_sb[:])

    nc.compile()
    return nc


_CACHE = {}


def get_nc():
    if "nc" not in _CACHE:
        _CACHE["nc"] = _build()
    return _CACHE["nc"]


def make_in_maps(x, Wq, bq, Wk, bk, Wv, bv):
    bf16 = ml_dtypes.bfloat16
    x = np.asarray(x, dtype=np.float32)
    Wq = np.asarray(Wq, dtype=np.float32)
    Wk = np.asarray(Wk, dtype=np.float32)
    Wv = np.asarray(Wv, dtype=np.float32)
    # weight blob: per c-chunk [Wq | Wv] then the 3 Wk chunks
    wp = np.zeros((128, 576), dtype=np.float32)
    for c in range(3):
        wp[:, c * 128 : c * 128 + 64] = Wq[c * 128 : (c + 1) * 128]
        wp[:, c * 128 + 64 : c * 128 + 128] = Wv[c * 128 : (c + 1) * 128]
        wp[:, 384 + c * 64 : 384 + (c + 1) * 64] = Wk[c * 128 : (c + 1) * 128]
    wp = np.ascontiguousarray(wp.astype(bf16))
    bias = np.zeros((128, 2), dtype=np.float32)
    bias[0:64, 0] = np.asarray(bq, dtype=np.float32).ravel()
    bias[64:128, 0] = np.asarray(bv, dtype=np.float32).ravel()
    bias[0:64, 1] = np.asarray(bk, dtype=np.float32).ravel()

    in_maps = []
    for i in range(N_CORES):
        shard = x[i * B_SHARD : (i + 1) * B_SHARD]  # [32, 256, 384]
        pairs = shard.reshape(NPAIR, 512, C)  # t within pair = bi*256 + t'
        # [b, p, k, t] with c = k*128 + p
        xT = pairs.transpose(0, 2, 1).reshape(NPAIR, 3, 128, 512).transpose(0, 2, 1, 3)
        xT = np.ascontiguousarray(xT.astype(bf16)).reshape(NPAIR * 128, 3 * 512)
        in_maps.append({"xT": xT, "wpack": wp, "bias": bias})
    return in_maps


def kernel(x, Wq, bq, Wk, bk, Wv, bv):
    nc = get_nc()
    in_maps = make_in_maps(x, Wq, bq, Wk, bk, Wv, bv)
    res = bass_utils.run_bass_kernel_spmd(nc, in_maps, core_ids=list(range(N_CORES)))
    outs = []
    for i in range(N_CORES):
        r = res.results[i]["out"].reshape(NPAIR, 128, 2, 2, 64)
        # [b, p, i, n, h] -> item 2b+i, t = n*128+p
        outs.append(
            np.ascontiguousarray(r.transpose(0, 2, 3, 1, 4)).reshape(B_SHARD, T, H)
        )
    return np.concatenate(outs, axis=0)


# revision 17
# speedup vs baseline: 1.6817x; 1.0325x over previous
"""Causal single-head attention on 8 trn2 cores, batch-data-parallel.

Computes, for each batch item b:
    Q = x[b] @ Wq + bq; K = x[b] @ Wk + bk; V = x[b] @ Wv + bv
    out[b] = softmax(causal_mask(Q K^T / sqrt(H))) @ V

Shapes: x [256, 256, 384], W* [384, 64], b* [64], out [256, 256, 64] fp32.
Sharding: batch axis split across 8 cores (32 items each), weights replicated.

All matmul operands are bfloat16 (PSUM accumulation stays fp32); rel err
~4e-3 against the fp32 reference (gate 2e-2).

Layout choices driven by trace analysis:
- Host feeds x^T per batch pair as [pair, partition, 3KB contiguous] so each
  pair is ONE dma with 128 descriptors (descriptor generation on the Sync
  engine at ~5ns/descriptor was a startup+steady bottleneck).
- All matmul weights ship in ONE packed dram tensor (12 dma_starts at ~0.7us
  of descriptor-gen each delayed the first x tile by ~8us in v2).
- Output is written pair-at-a-time in a [pair, partition, item, half, h]
  layout (1KB/partition contiguous, one dma per pair); the host undoes it.

Per pair (batch items 2bp, 2bp+1):
  qv psum [128,512] = rows 0:64 Q^T, rows 64:128 V^T (lhsT=[Wq|Wv], rhs=x^T)
  k  psum [64,512]  = K^T
  qv2/k2 sbuf (bf16) = psum + per-partition bias
  scores^T psum [128, item, 3, 128]: per item blocks (s0,t0) (s0,t1) (s1,t1);
      the causally-dead (s1,t0) block is never computed.
  W = exp(SCALE*scores^T) in ONE activation op (psum -> bf16 sbuf);
      the causal mask is applied AFTER exp by zeroing the upper triangle of
      the two diagonal blocks per item with gpsimd affine_select (GpSimd has
      no PSUM port, but W is in SBUF; this keeps mask work off the busy DVE).
  V' = [V | 1 | 1] natural layout via PE transposes of V^T; the ones columns
       make the out matmul also produce the softmax denominator (wei @ 1)
  out' = wei^T.T @ V' -> [t, 64 | den | den];  out = out'[:,0:64] * (1/den)
"""

import ml_dtypes
import numpy as np

import concourse.bacc as bacc
import concourse.mybir as mybir
import concourse.tile as tile
from concourse import bass_utils
from concourse.masks import make_identity

N_CORES = 8
B_FULL, T, C, H = 256, 256, 384, 64
B_SHARD = B_FULL // N_CORES  # 32
NPAIR = B_SHARD // 2  # 16
F32 = mybir.dt.float32
BF16 = mybir.dt.bfloat16
SCALE = float(H) ** -0.5

EXP = mybir.ActivationFunctionType.Exp


def _build():
    MMT = BF16
    nc = bacc.Bacc("TRN2", target_bir_lowering=False, debug=False, num_devices=N_CORES)

    # x^T pair-major: row (bp*128+p), cols (k*512+t) -> x[pair bp][t][k*128+p]
    xT_d = nc.dram_tensor("xT", [NPAIR * 128, 3 * 512], MMT, kind="ExternalInput").ap()
    # all matmul weights in one blob: cols [c*128 .. c*128+64) = Wq chunk c,
    # [c*128+64 ..) = Wv chunk c, cols [384+c*64) = Wk chunk c
    wp_d = nc.dram_tensor("wpack", [128, 576], MMT, kind="ExternalInput").ap()
    # biases: col 0 = [bq;bv], col 1 = [bk;0]
    bias_d = nc.dram_tensor("bias", [128, 2], F32, kind="ExternalInput").ap()
    # out pair-major: row (bp*128+p), cols ((bi*2+n)*64+h) -> item 2bp+bi,
    # t = n*128+p
    out_d = nc.dram_tensor("out", [NPAIR * 128, 256], MMT, kind="ExternalOutput").ap()

    xT_r = xT_d.rearrange("(b p) (k t) -> b p k t", p=128, k=3)
    out_r = out_d.rearrange("(b p) (i n h) -> b p i n h", p=128, i=2, n=2)

    with tile.TileContext(nc) as tc:
        with (
            tc.tile_pool(name="singles", bufs=1) as singles,
            tc.tile_pool(name="sb", bufs=3) as sb,
            tc.tile_pool(name="sbx", bufs=3) as sbx,
            tc.tile_pool(name="ps_qv", bufs=2, space="PSUM") as ps_qv,
            tc.tile_pool(name="ps_k", bufs=1, space="PSUM") as ps_k,
            tc.tile_pool(name="ps_s", bufs=1, space="PSUM") as ps_s,
            tc.tile_pool(name="ps_v", bufs=1, space="PSUM") as ps_v,
            tc.tile_pool(name="ps_o", bufs=2, space="PSUM") as ps_o,
        ):
            # ---- one-time setup ----
            wt = singles.tile([128, 576], MMT)
            nc.sync.dma_start(wt[:], wp_d[:])
            bias_t = singles.tile([128, 2], F32)
            nc.sync.dma_start(bias_t[:], bias_d[:])
            wqv = wt[:, 0:384].rearrange("p (c m) -> p c m", c=3)  # [128, 3, 128]
            wkk = wt[:, 384:576].rearrange("p (c m) -> p c m", c=3)  # [128, 3, 64]
            bqv_t = bias_t[:, 0:1]
            bkk_t = bias_t[:, 1:2]  # [bk; bk] (k psum holds both t-halves)

            identf = singles.tile([128, 128], F32)
            make_identity(nc, identf[:])
            ident = singles.tile([128, 128], MMT)
            nc.vector.tensor_copy(ident[:], identf[:])

            for bp in range(NPAIR):
                # x^T for the pair: [p, k, t] (3KB/partition, one dma)
                xt = sbx.tile([128, 3, 512], MMT, tag="xt")
                nc.sync.dma_start(xt[:], xT_r[bp])

                # pair projections
                qv_ps = ps_qv.tile([128, 512], F32, tag="qv_ps")
                k_ps = ps_k.tile([64, 512], F32, tag="k_ps")
                for c in range(3):
                    nc.tensor.matmul(
                        qv_ps[:], wqv[:, c, :], xt[:, c, :], start=(c == 0), stop=(c == 2)
                    )
                for c in range(3):
                    nc.tensor.matmul(
                        k_ps[:], wkk[:, c, :], xt[:, c, :], start=(c == 0), stop=(c == 2)
                    )
                qv2 = sb.tile([128, 512], MMT, tag="qv2")
                k2 = sb.tile([64, 512], MMT, tag="k2")
                nc.vector.tensor_single_scalar(
                    qv2[:], qv_ps[:], bqv_t, op=mybir.AluOpType.add
                )
                nc.vector.tensor_single_scalar(
                    k2[:], k_ps[:], bkk_t[0:64, :], op=mybir.AluOpType.add
                )

                # scores^T per item: blocks 0=(s0,t0) 1=(s0,t1) 2=(s1,t1);
                # item stride padded to 2048B so each matmul stays in one bank
                W = sb.tile([128, 2, 3, 128], MMT, tag="W")
                s_ps = ps_s.tile([128, 2, 4, 128], F32, tag="s_ps")
                for bi in range(2):
                    toff = bi * 256
                    qT = qv2[0:64, toff : toff + 256]
                    kT = k2[0:64, toff : toff + 256]
                    nc.tensor.matmul(
                        s_ps[:, bi, 0:2, :], kT[:, 0:128], qT, start=True, stop=True
                    )
                    nc.tensor.matmul(
                        s_ps[:, bi, 2, :], kT[:, 128:256], qT[:, 128:256], start=True, stop=True
                    )
                # wei^T = exp(SCALE*scores^T), ONE op per pair, bf16 to SBUF
                nc.scalar.activation(W[:], s_ps[:, :, 0:3, :], EXP, scale=SCALE)
                # causal mask: zero upper triangle of the diagonal blocks
                for bi in range(2):
                    for j in (0, 2):
                        nc.gpsimd.affine_select(
                            out=W[:, bi, j, :],
                            in_=W[:, bi, j, :],
                            compare_op=mybir.AluOpType.is_ge,
                            fill=0.0,
                            base=0,
                            pattern=[[1, 128]],  # keep where (-s + t) >= 0
                            channel_multiplier=-1,
                        )

                # V natural [s, h] for both items via PE transposes of V^T
                v_ps = ps_v.tile([128, 4, 64], MMT, tag="v_ps")
                for q in range(4):
                    nc.tensor.transpose(
                        v_ps[:, q, :],
                        qv2[64:128, q * 128 : (q + 1) * 128],
                        ident[64:128, 64:128],
                    )
                v_sb = sb.tile([128, 2, 2, 66], MMT, tag="v_sb", bufs=3)
                nc.scalar.copy(
                    v_sb[:, :, :, 0:64].rearrange("p i n h -> p (i n) h"), v_ps[:]
                )
                if bp < 3:
                    # slots rotate round-robin; the copy above only writes
                    # cols 0:64, so the ones columns survive slot reuse
                    nc.vector.memset(v_sb[:, :, :, 64:66], 1.0)

                # out' = wei^T.T @ [V|1|1] -> [t, 64 | den | den]
                o_ps = ps_o.tile([128, 2, 2, 66], F32, tag="o_ps")
                for bi in range(2):
                    nc.tensor.matmul(
                        o_ps[:, bi, 0, :], W[:, bi, 0, :], v_sb[:, bi, 0, :],
                        start=True, stop=True,
                    )
                    nc.tensor.matmul(
                        o_ps[:, bi, 1, :], W[:, bi, 1, :], v_sb[:, bi, 0, :],
                        start=True, stop=False,
                    )
                    nc.tensor.matmul(
                        o_ps[:, bi, 1, :], W[:, bi, 2, :], v_sb[:, bi, 1, :],
                        start=False, stop=True,
                    )

                rden = sb.tile([128, 2, 2, 1], F32, tag="rden")
                nc.vector.reciprocal(rden[:, :, :, 0], o_ps[:, :, :, 64])
                o_sb = sb.tile([128, 2, 2, 64], MMT, tag="o_sb")
                nc.vector.tensor_mul(
                    o_sb[:],
                    o_ps[:, :, :, 0:64],
                    rden[:].broadcast_to([128, 2, 2, 64]),
                )
                nc.sync.dma_start(out_r[bp], o_sb[:])

    nc.compile()
    return nc


_CACHE = {}


def get_nc():
    if "nc" not in _CACHE:
        _CACHE["nc"] = _build()
    return _CACHE["nc"]


def make_in_maps(x, Wq, bq, Wk, bk, Wv, bv):
    bf16 = ml_dtypes.bfloat16
    x = np.asarray(x, dtype=np.float32)
    Wq = np.asarray(Wq, dtype=np.float32)
    Wk = np.asarray(Wk, dtype=np.float32)
    Wv = np.asarray(Wv, dtype=np.float32)
    # weight blob: per c-chunk [Wq | Wv] then the 3 Wk chunks
    wp = np.zeros((128, 576), dtype=np.float32)
    for c in range(3):
        wp[:, c * 128 : c * 128 + 64] = Wq[c * 128 : (c + 1) * 128]
        wp[:, c * 128 + 64 : c * 128 + 128] = Wv[c * 128 : (c + 1) * 128]
        wp[:, 384 + c * 64 : 384 + (c + 1) * 64] = Wk[c * 128 : (c + 1) * 128]
    wp = np.ascontiguousarray(wp.astype(bf16))
    bias = np.zeros((128, 2), dtype=np.float32)
    bias[0:64, 0] = np.asarray(bq, dtype=np.float32).ravel()
    bias[64:128, 0] = np.asarray(bv, dtype=np.float32).ravel()
    bias[0:64, 1] = np.asarray(bk, dtype=np.float32).ravel()
    bias[64:128, 1] = np.asarray(bk, dtype=np.float32).ravel()

    in_maps = []
    for i in range(N_CORES):
        shard = x[i * B_SHARD : (i + 1) * B_SHARD]  # [32, 256, 384]
        pairs = shard.reshape(NPAIR, 512, C)  # t within pair = bi*256 + t'
        # [b, p, k, t] with c = k*128 + p
        xT = pairs.transpose(0, 2, 1).reshape(NPAIR, 3, 128, 512).transpose(0, 2, 1, 3)
        xT = np.ascontiguousarray(xT.astype(bf16)).reshape(NPAIR * 128, 3 * 512)
        in_maps.append({"xT": xT, "wpack": wp, "bias": bias})
    return in_maps


def kernel(x, Wq, bq, Wk, bk, Wv, bv):
    nc = get_nc()
    in_maps = make_in_maps(x, Wq, bq, Wk, bk, Wv, bv)
    res = bass_utils.run_bass_kernel_spmd(nc, in_maps, core_ids=list(range(N_CORES)))
    outs = []
    for i in range(N_CORES):
        r = res.results[i]["out"].reshape(NPAIR, 128, 2, 2, 64).astype(np.float32)
        # [b, p, i, n, h] -> item 2b+i, t = n*128+p
        outs.append(
            np.ascontiguousarray(r.transpose(0, 2, 3, 1, 4)).reshape(B_SHARD, T, H)
        )
    return np.concatenate(outs, axis=0)
